# revision 4
# baseline (speedup 1.0000x reference)
"""Single-head memory attention on Trainium2, batch-parallel across 8 NeuronCores.

Host-side prep (per batch element): transpose x/keys/Wq and cast everything
to bf16, so the device kernel is pure matmul pipeline — no PE transposes,
no on-chip casts, half the DMA bytes.

Per core (one batch element), all operands bf16, fp32 PSUM accumulation:
    QT = WqT.T @ xT + bq                  (MM1: contraction d on partitions)
    ST = keysT.T @ QT                     (MM2: contraction e on partitions)
    ET = exp(ST/sqrt(d) + mask_k)         (one ScalarE activation per kt tile)
    MM3 streams V in 4 groups of 257 cols: cols 0..255 are V, col 256 is a
    ones column, so each PSUM group lands [O_part | sum_k E] and the softmax
    denominator falls out of the same accumulation — no separate ones pass,
    no partition-scatter of the sums row.
    O = (E.T @ V) * recip(denominator)    (per-partition normalize, bf16 out)
"""

import numpy as np
import ml_dtypes

import concourse.bacc as bacc
import concourse.mybir as mybir
from concourse.tile import TileContext
from concourse.bass_utils import run_bass_kernel_spmd

B, LQ, LK, D = 8, 2048, 2048, 1024
P = 128
QCH = 512                 # queries processed per chunk
NQC = LQ // QCH           # 4 chunks
NDT = D // P              # 8 tiles along d (contraction of MM1)
NET = D // P              # 8 tiles along e (contraction of MM2)
NKT = LK // P             # 16 tiles along k (contraction of MM3)
NQS = QCH // P            # 4 query subtiles per chunk
GW = 256                  # MM3 value-column group width
NG = D // GW              # 4 groups; each streams GW V cols + 1 ones col
SCALE = 1.0 / float(np.sqrt(D))

F32 = mybir.dt.float32
BF16 = mybir.dt.bfloat16
AFT = mybir.ActivationFunctionType

_CACHE = {}


def build_nc():
    nc = bacc.Bacc(None, target_bir_lowering=False)

    xT_d = nc.dram_tensor("xT", [D, LQ], BF16, kind="ExternalInput")
    keysT_d = nc.dram_tensor("keysT", [D, LK], BF16, kind="ExternalInput")
    values_d = nc.dram_tensor("values", [LK, D], BF16, kind="ExternalInput")
    wqT_d = nc.dram_tensor("WqT", [D, D], BF16, kind="ExternalInput")
    mask_d = nc.dram_tensor("mask", [LK, 1], F32, kind="ExternalInput")
    bq_d = nc.dram_tensor("bq", [D], F32, kind="ExternalInput")
    out_d = nc.dram_tensor("out", [LQ, D], BF16, kind="ExternalOutput")

    with TileContext(nc) as tc:
        with (
            tc.tile_pool(name="persist", bufs=1) as persist,
            tc.tile_pool(name="xTp", bufs=2) as xTp,
            tc.tile_pool(name="QTp", bufs=2) as QTp,
            tc.tile_pool(name="ETp", bufs=2) as ETp,
            tc.tile_pool(name="osb", bufs=3) as osbp,
            tc.tile_pool(name="rcp", bufs=4) as rcp,
            tc.tile_pool(name="psAcc", bufs=5, space="PSUM") as psAccp,
            tc.tile_pool(name="psO", bufs=3, space="PSUM") as psOp,
        ):
            # ---- persistent operands ----
            WqT = persist.tile([P, NDT, D], BF16)     # [d%P, d//P, e] = Wq[e, d]
            keysT = persist.tile([P, NET, LK], BF16)  # [e%P, e//P, k] = keys[k, e]
            # Vaug[:, kt, g, 0:GW] = values rows, Vaug[:, kt, g, GW] = 1.0
            Vaug = persist.tile([P, NKT, NG, GW + 1], BF16)
            bq_sb = persist.tile([P, NDT], F32)
            mask_sb = persist.tile([P, NKT], F32)

            nc.any.memset(Vaug[:, :, :, GW:GW + 1], 1.0)

            def x_stage(qc):
                xT = xTp.tile([P, NDT, QCH], BF16, tag="xT")
                for dt in range(NDT):
                    nc.sync.dma_start(
                        xT[:, dt, :],
                        xT_d[dt * P:(dt + 1) * P, qc * QCH:(qc + 1) * QCH],
                    )
                return xT

            def mm1(xT):
                # QT[e, q] = Wq @ x^T + bq
                QT = QTp.tile([P, NET, QCH], BF16, tag="QT")
                for et in range(NET):
                    pq = psAccp.tile([P, QCH], F32, tag="acc")
                    for dt in range(NDT):
                        nc.tensor.matmul(
                            pq,
                            WqT[:, dt, et * P:(et + 1) * P],
                            xT[:, dt, :],
                            start=(dt == 0),
                            stop=(dt == NDT - 1),
                        )
                    nc.vector.tensor_scalar_add(QT[:, et, :], pq, bq_sb[:, et:et + 1])
                return QT

            def mm2(QT):
                # ST[k, q] = keys @ Q^T ; ET = exp(ST/sqrt(d) + mask_k)
                ET = ETp.tile([P, NKT, QCH], BF16, tag="ET")
                for kt in range(NKT):
                    ps = psAccp.tile([P, QCH], F32, tag="acc")
                    for et in range(NET):
                        nc.tensor.matmul(
                            ps,
                            keysT[:, et, kt * P:(kt + 1) * P],
                            QT[:, et, :],
                            start=(et == 0),
                            stop=(et == NET - 1),
                        )
                    nc.scalar.activation(
                        ET[:, kt, :], ps, AFT.Exp,
                        bias=mask_sb[:, kt:kt + 1], scale=SCALE,
                    )
                return ET

            def mm3(qc, ET):
                # O[q, dv] = sum_k E[k,q] Vaug[k,dv]; col GW of each group is
                # the denominator; normalize with its reciprocal.
                for qs in range(NQS):
                    osb = osbp.tile([P, D], BF16, tag="osb")
                    rc = rcp.tile([P, 1], F32, tag="rc")
                    for g in range(NG):
                        po = psOp.tile([P, GW + 1], F32, tag="po")
                        for kt in range(NKT):
                            nc.tensor.matmul(
                                po,
                                ET[:, kt, qs * P:(qs + 1) * P],
                                Vaug[:, kt, g, :],
                                start=(kt == 0),
                                stop=(kt == NKT - 1),
                            )
                        if g == 0:
                            nc.vector.reciprocal(rc, po[:, GW:GW + 1])
                        oslice = osb[:, g * GW:(g + 1) * GW]
                        if g % 2 == 0:
                            nc.vector.tensor_scalar_mul(oslice, po[:, 0:GW], rc)
                        else:
                            nc.scalar.activation(
                                oslice, po[:, 0:GW], AFT.Copy,
                                bias=0.0, scale=rc,
                            )
                    nc.sync.dma_start(
                        out_d[qc * QCH + qs * P: qc * QCH + (qs + 1) * P, :],
                        osb,
                    )

            # ---- emission ----
            for dt in range(NDT):
                nc.sync.dma_start(WqT[:, dt, :], wqT_d[dt * P:(dt + 1) * P, :])
            xT_next = x_stage(0)
            nc.sync.dma_start(bq_sb, bq_d[:].rearrange("(t p) -> p t", p=P))
            nc.sync.dma_start(
                mask_sb, mask_d[:].rearrange("(t p) o -> p (t o)", p=P)
            )
            for et in range(NET):
                nc.sync.dma_start(
                    keysT[:, et, :], keysT_d[et * P:(et + 1) * P, :]
                )
            for kt in range(NKT):
                nc.sync.dma_start(
                    Vaug[:, kt, :, 0:GW],
                    values_d[kt * P:(kt + 1) * P, :].rearrange(
                        "p (g c) -> p g c", g=NG
                    ),
                )
            for qc in range(NQC):
                xT = xT_next
                QT = mm1(xT)
                ET = mm2(QT)
                if qc + 1 < NQC:
                    xT_next = x_stage(qc + 1)
                mm3(qc, ET)

    nc.finalize()
    return nc


def _get_nc():
    if "nc" not in _CACHE:
        _CACHE["nc"] = build_nc()
    return _CACHE["nc"]


def _prep(x, mem_padding_mask, keys, values, Wq, bq):
    bf = ml_dtypes.bfloat16
    WqT_c = np.ascontiguousarray(np.asarray(Wq, dtype=np.float32).T.astype(bf))
    bq_c = np.ascontiguousarray(bq, dtype=np.float32)
    maps = []
    for b in range(B):
        maps.append({
            "xT": np.ascontiguousarray(
                np.asarray(x[b], dtype=np.float32).T.astype(bf)),
            "keysT": np.ascontiguousarray(
                np.asarray(keys[b], dtype=np.float32).T.astype(bf)),
            "values": np.ascontiguousarray(
                np.asarray(values[b], dtype=np.float32).astype(bf)),
            "mask": np.ascontiguousarray(mem_padding_mask[b], dtype=np.float32),
            "WqT": WqT_c,
            "bq": bq_c,
        })
    return maps


def kernel(x, mem_padding_mask, keys, values, Wq, bq):
    nc = _get_nc()
    in_maps = _prep(x, mem_padding_mask, keys, values, Wq, bq)
    res = run_bass_kernel_spmd(nc, in_maps, core_ids=list(range(B)))
    return np.stack(
        [res.results[i]["out"] for i in range(B)], axis=0
    ).astype(np.float32)
